# revision 45
# baseline (speedup 1.0000x reference)
"""NestedAttention Trainium2 kernel (v6).

Reference computation (per batch b):
  q_i = wq[i] @ x ; k_j = wk[j] @ x ; v_j = wv[j] @ x        (1x1 convs, r=64)
  for i: acc_i = sum_j softmax_m(q_i^T k_j / sqrt(r)) applied to v_j
  out = wo @ concat_i(acc_i) ; y = x * sigmoid(out)

Sharding: 8 cores = batch(4) x query-column-halves(2). Each core holds full
k/v (m = 2304 keys) and a 1152-wide slice of query columns n; no cross-core
communication is needed (softmax is over m, fully on-core).

Design (v6):
  * exp split between ScalarE (exact table exp) and VectorE (Schraudolph
    bit-trick: one tensor_scalar f32->int8 writes fp8e4m3 exp bits directly);
    softmax normalization cancels the sawtooth error (~5e-4 end-to-end).
  * mm2 (v @ E with a ones-block computing Z) in fp8 DoubleRow: E and vT are
    fp8e4m3, each pass contracts two m-tiles (9 passes), interleaved 3 per
    mm1 slot, chunk-major, one pair behind mm1.
  * logit PSUM: [128,1024] tiles x3 bufs (exactly 2 banks each -> 3-deep
    exp pipelining) + the 128-wide n-tail of 4 m-tiles batched into one
    shared bank, exp'd in one ScalarE op.
  * q/k stored duplicated across both partition halves (doubled projection
    weights, exp scale halved) - the K=128 matmul then needs no zero pad.
  * softmax denominator: partition-shifted Z copy (DVE or ScalarE), in-place
    reciprocal_approx_fast; the j>0 accumulate runs on GPSIMD.
"""

import math
import os
import numpy as np

B, C, H, W = 4, 256, 48, 48
N = H * W            # 2304 keys (m) per image
NSLICE = N // 2      # 1152 query columns (n) per core
R = 64               # reduced channels
P = 128
MT = N // P          # 18 m-tiles
KT = C // P          # 2 contraction tiles over channels
MAIN = 1024          # main logit block; 1152 = 1024 + 128 runt
CHUNKS = [(0, 512), (512, 512), (1024, 128)]  # mm2 n chunks, bank aligned
N_CORES = 8

# m-tiles whose main exp runs on the DVE (Schraudolph); rest on ScalarE.
# Split found by on-hardware search; balances ScalarE/VectorE occupancy.
DVE_EXP_MTS = frozenset(
    int(s)
    for s in os.environ.get("NESTED_DVE_MTS", "1,3,5,7,9,13,15").split(",")
    if s
)

NORM_MODE = os.environ.get("NESTED_NORM", "copy1")  # copy1 | acopy
ADD_ENGINE = os.environ.get("NESTED_ADD", "gpsimd")  # gpsimd | vector

# Schraudolph: fp8e4m3_bits = round(logit * scale * 8/ln2 + (7*8 - sigma)).
# q/k are duplicated across both partition halves so the K=128 dot product
# double-counts; the extra 0.5 compensates.
SCH_C1 = 0.5 * 0.125 * 8.0 / math.log(2.0)
SCH_C2 = 7.0 * 8.0 - 0.5
EXP_SCALE = 0.5 * 0.125

_CACHE = {}
LAST_RESULTS = None


def _build_program():
    from contextlib import ExitStack

    import concourse.bass as bass
    import concourse.tile as tile
    from concourse import bacc, mybir

    f32 = mybir.dt.float32
    bf16 = mybir.dt.bfloat16
    fp8 = mybir.dt.float8e4
    i8 = mybir.dt.int8
    Exp = mybir.ActivationFunctionType.Exp
    Tanh = mybir.ActivationFunctionType.Tanh
    mult = mybir.AluOpType.mult
    add = mybir.AluOpType.add
    DR = mybir.MatmulPerfMode.DoubleRow

    nc = bacc.Bacc("TRN2", target_bir_lowering=False, debug=False)
    xb_d = nc.declare_dram_parameter("xb", [KT, P, N], bf16, isOutput=False)
    xnb_d = nc.declare_dram_parameter("xnb", [KT, P, NSLICE], bf16, isOutput=False)
    xn_d = nc.declare_dram_parameter("xn", [KT, P, NSLICE], f32, isOutput=False)
    wqT_d = nc.declare_dram_parameter("wqT", [KT, P, 3, 2 * R], bf16, isOutput=False)
    wkT_d = nc.declare_dram_parameter("wkT", [KT, P, 3, 2 * R], bf16, isOutput=False)
    wvT_d = nc.declare_dram_parameter("wvT", [KT, P, 3, R], bf16, isOutput=False)
    woT_d = nc.declare_dram_parameter("woT", [3, R, C], bf16, isOutput=False)
    y_d = nc.declare_dram_parameter("y", [KT, P, NSLICE], f32, isOutput=True)

    with tile.TileContext(nc) as tc, ExitStack() as ctx:
        consts = ctx.enter_context(tc.tile_pool(name="consts", bufs=1))
        # PSUM: 3x2 banks (main logits) + 1 bank (runts) + 1 bank (mm2) = 8
        big_ps = ctx.enter_context(tc.tile_pool(name="big_ps", bufs=3, space="PSUM"))
        runt_ps = ctx.enter_context(tc.tile_pool(name="runt_ps", bufs=1, space="PSUM"))
        mm2_ps = ctx.enter_context(tc.tile_pool(name="mm2_ps", bufs=1, space="PSUM"))
        e_pool = ctx.enter_context(tc.tile_pool(name="e_pool", bufs=3))
        rb_pool = ctx.enter_context(tc.tile_pool(name="rb_pool", bufs=4))
        small = ctx.enter_context(tc.tile_pool(name="small", bufs=4))

        # ---- persistent SBUF state ----
        wqT_sb = consts.tile([P, KT, 3, 2 * R], bf16)
        nc.sync.dma_start(wqT_sb[:], wqT_d.rearrange("t p i r -> p t i r"))
        xnb_sb = consts.tile([P, KT, NSLICE], bf16)
        nc.sync.dma_start(xnb_sb[:], xnb_d.rearrange("t p m -> p t m"))
        wkT_sb = consts.tile([P, KT, 3, 2 * R], bf16)
        nc.sync.dma_start(wkT_sb[:], wkT_d.rearrange("t p i r -> p t i r"))
        x_sb = consts.tile([P, KT, N], bf16)
        nc.sync.dma_start(
            x_sb[:, :, 0:NSLICE], xb_d[:, :, 0:NSLICE].rearrange("t p m -> p t m")
        )
        nc.sync.dma_start(
            x_sb[:, :, NSLICE:N], xb_d[:, :, NSLICE:N].rearrange("t p m -> p t m")
        )
        wvT_sb = consts.tile([P, KT, 3, R], bf16)
        nc.sync.dma_start(wvT_sb[:], wvT_d.rearrange("t p i r -> p t i r"))
        xn_sb = consts.tile([P, KT, NSLICE], f32)
        nc.sync.dma_start(xn_sb[:], xn_d.rearrange("t p m -> p t m"))

        # Warm the ScalarE exp table set immediately so the ~2.7us table
        # load overlaps the input DMAs instead of stalling the first exp.
        warm = consts.tile([P, 520], bf16)
        nc.gpsimd.memset(warm[:], 0.0)
        nc.scalar.activation(
            warm[:, 0:8].bitcast(f32), warm[:, 8:16].bitcast(f32), Exp
        )

        woT_sb = []
        for i in range(3):
            w = consts.tile([P, C], bf16, tag=f"woT{i}")
            nc.gpsimd.memset(w[R:P, :], 0.0)
            nc.sync.dma_start(w[0:R, :], woT_d[i])
            woT_sb.append(w)

        # q/k live duplicated: partitions 0:64 and 64:128 hold the same values
        q_sb = consts.tile([P, 3, NSLICE], bf16)
        k_sb = consts.tile([P, 3, N], bf16)

        # vT buffer per m-tile (fp8): [vT_0 ones | vT_1 ones | vT_2 ones]
        vT_buf = consts.tile([P, MT, 384], fp8)
        for j in range(3):
            nc.gpsimd.memset(vT_buf[:, :, 128 * j + 64 : 128 * j + 128], 1.0)

        # acc_i accumulated in bf16; rows 64-127 zero (K=128 pad for final mm)
        acc = []
        for i in range(3):
            a = consts.tile([P, NSLICE], bf16, tag=f"acc{i}")
            nc.gpsimd.memset(a[R:P, :], 0.0)
            acc.append(a)

        # ---- projections (weights doubled -> output rows 0:64 == 64:128) ----
        def emit_proj(dst, w_sb, src_sb, idx3, src_off):
            ptm = big_ps.tile([P, MAIN], f32, tag="big")
            ptt = mm2_ps.tile([P, 512], f32, tag="mm2")
            for kt in range(KT):
                for c0, cw in ((0, 512), (512, 512)):
                    nc.tensor.matmul(
                        ptm[:, c0 : c0 + cw],
                        w_sb[:, kt, idx3, :],
                        src_sb[:, kt, src_off + c0 : src_off + c0 + cw],
                        start=(kt == 0),
                        stop=(kt == KT - 1),
                    )
                nc.tensor.matmul(
                    ptt[:, 0:128],
                    w_sb[:, kt, idx3, :],
                    src_sb[:, kt, src_off + 1024 : src_off + 1152],
                    start=(kt == 0),
                    stop=(kt == KT - 1),
                )
            nc.vector.tensor_copy(dst[:, 0:MAIN], ptm[:])
            nc.vector.tensor_copy(dst[:, MAIN:NSLICE], ptt[:, 0:128])

        def emit_q(i):
            emit_proj(q_sb[:, i, :], wqT_sb, xnb_sb, i, 0)

        def emit_k(j, half):
            emit_proj(
                k_sb[:, j, half * NSLICE : (half + 1) * NSLICE],
                wkT_sb,
                x_sb,
                j,
                half * NSLICE,
            )

        def emit_vT(mts):
            for mt in mts:
                pv = mm2_ps.tile([P, 512], f32, tag="mm2")
                for kt in range(KT):
                    nc.tensor.matmul(
                        pv[:, 0 : 3 * R],
                        x_sb[:, kt, mt * P : (mt + 1) * P],
                        wvT_sb[:, kt, :, :],
                        start=(kt == 0),
                        stop=(kt == KT - 1),
                    )
                base = vT_buf[:, mt, :]
                dst = bass.AP(
                    tensor=base.tensor,
                    offset=base.offset,
                    ap=[base.ap[0], [128, 3], [1, R]],
                )
                nc.vector.tensor_copy(
                    dst, pv[:, 0 : 3 * R].rearrange("p (j r) -> p j r", j=3)
                )

        # ---- attention pair pipeline ----
        def emit_exp_main(E, mt, pt):
            if mt in DVE_EXP_MTS:
                nc.vector.tensor_scalar(
                    E[:, mt, 0:MAIN].bitcast(i8), pt[:], SCH_C1, SCH_C2, mult, add
                )
            else:
                nc.scalar.activation(E[:, mt, 0:MAIN], pt[:], Exp, scale=EXP_SCALE)

        def emit_exp_runt(E, rt, mt0, nmt):
            nc.scalar.activation(
                E[:, mt0 : mt0 + nmt, MAIN:NSLICE],
                rt[:, 0 : nmt * 128].rearrange("p (g r) -> p g r", r=128),
                Exp,
                scale=EXP_SCALE,
            )

        def emit_mm1_slot(E, i, j, ms, rt):
            # Main matmuls + exps first; the runt matmuls go last so their
            # WAR on the single runt bank (previous group's ScalarE exp) has
            # a full slot of extra slack and never blocks the PE FIFO.
            for mt in (2 * ms, 2 * ms + 1):
                pt = big_ps.tile([P, MAIN], f32, tag="big")
                for c0, cw in ((0, 512), (512, 512)):
                    nc.tensor.matmul(
                        pt[:, c0 : c0 + cw],
                        k_sb[:, j, mt * P : (mt + 1) * P],
                        q_sb[:, i, c0 : c0 + cw],
                        start=True,
                        stop=True,
                    )
                emit_exp_main(E, mt, pt)
            for mt in (2 * ms, 2 * ms + 1):
                r0 = 128 * (mt % 4)
                nc.tensor.matmul(
                    rt[:, r0 : r0 + 128],
                    k_sb[:, j, mt * P : (mt + 1) * P],
                    q_sb[:, i, MAIN:NSLICE],
                    start=True,
                    stop=True,
                )
                if mt % 4 == 3:
                    emit_exp_runt(E, rt, mt - 3, 4)
                elif mt == MT - 1:
                    emit_exp_runt(E, rt, mt - 1, 2)

        po_m = [None, None]
        po_t = [None]

        def emit_final_chunk(c0, cw):
            for mtile in range(KT):
                dst = (
                    po_m[mtile][:, c0 : c0 + cw]
                    if c0 < MAIN
                    else po_t[0][:, 128 * mtile : 128 * mtile + cw]
                )
                for i in range(3):
                    nc.tensor.matmul(
                        dst,
                        woT_sb[i][:, mtile * P : (mtile + 1) * P],
                        acc[i][:, c0 : c0 + cw],
                        start=(i == 0),
                        stop=(i == 2),
                    )
            for mtile in range(KT):
                src = (
                    po_m[mtile][:, c0 : c0 + cw]
                    if c0 < MAIN
                    else po_t[0][:, 128 * mtile : 128 * mtile + cw]
                )
                # sigmoid(x) = 0.5*(1 + tanh(x/2)); tanh shares the ACT table
                # set with exp (no mid-kernel table switch). xn_sb holds x/2.
                sig = small.tile([P, 512], f32, tag="sig")
                nc.scalar.activation(sig[:, 0:cw], src, Tanh, scale=0.5)
                y_sb = small.tile([P, 512], f32, tag="ysb")
                nc.vector.scalar_tensor_tensor(
                    y_sb[:, 0:cw],
                    sig[:, 0:cw],
                    1.0,
                    xn_sb[:, mtile, c0 : c0 + cw],
                    add,
                    mult,
                )
                nc.sync.dma_start(y_d[mtile][:, c0 : c0 + cw], y_sb[:, 0:cw])

        def emit_mm2_pass(pa, j, E, ks, c0, cw):
            nc.tensor.matmul(
                pa[:, 0:cw],
                vT_buf[:, 2 * ks : 2 * ks + 2, 128 * j : 128 * (j + 1)],
                E[:, 2 * ks : 2 * ks + 2, c0 : c0 + cw],
                start=(ks == 0),
                stop=(ks == MT // 2 - 1),
                perf_mode=DR,
            )

        def emit_norm(pa, i, j, c0, cw, last=False):
            rb = rb_pool.tile([R, 512], f32, tag="rb")
            # j==2 norms run while ScalarE has slack (the tail region):
            # route their Z copy through ScalarE to shorten the DVE chain.
            if NORM_MODE == "acopy" or j == 2:
                nc.scalar.copy(rb[:, 0:cw], pa[R:P, 0:cw])
            else:
                nc.vector.tensor_copy(rb[:, 0:cw], pa[R:P, 0:cw])
            nc.vector.reciprocal_approx_fast(rb[:, 0:cw], rb[:, 0:cw])
            if j == 0:
                nc.vector.tensor_tensor(
                    acc[i][0:R, c0 : c0 + cw], pa[0:R, 0:cw], rb[:, 0:cw], mult
                )
            else:
                tmp = small.tile([R, 512], bf16, tag="tmp")
                nc.vector.tensor_tensor(
                    tmp[:, 0:cw], pa[0:R, 0:cw], rb[:, 0:cw], mult
                )
                eng = nc.gpsimd if ADD_ENGINE == "gpsimd" else nc.vector
                eng.tensor_tensor(
                    acc[i][0:R, c0 : c0 + cw],
                    acc[i][0:R, c0 : c0 + cw],
                    tmp[:, 0:cw],
                    add,
                )
            if last:
                emit_final_chunk(c0, cw)

        def emit_mm2_norm_tail(i, j, E):
            # Epilogue for the last pair: keep the PE dense by giving each
            # chunk its own PSUM bank (big pool + mm2 bank are free now) so
            # all 27 DR passes run back-to-back while norms/finals overlap.
            pa0 = big_ps.tile([P, MAIN], f32, tag="big", name="pa_ep0")
            for ks in range(MT // 2):
                emit_mm2_pass(pa0, j, E, ks, 0, 512)
            emit_norm(pa0, i, j, 0, 512)
            pa1 = big_ps.tile([P, MAIN], f32, tag="big", name="pa_ep1")
            for ks in range(MT // 2):
                emit_mm2_pass(pa1, j, E, ks, 512, 512)
            emit_norm(pa1, i, j, 512, 512)
            # chunk-2 accumulator and the final's n-tail share the runt bank,
            # which frees early (no dependency on the previous pair's norms).
            rt_ep = runt_ps.tile([P, 512], f32, tag="runt", name="rt_ep")
            pa2 = rt_ep
            for ks in range(MT // 2):
                emit_mm2_pass(pa2, j, E, ks, 1024, 128)
            emit_norm(pa2, i, j, 1024, 128)
            po_m[0] = big_ps.tile([P, MAIN], f32, tag="big", name="po0")
            po_m[1] = big_ps.tile([P, MAIN], f32, tag="big", name="po1")
            po_t[0] = rt_ep[:, 128:512]
            emit_final_chunk(0, 512)
            emit_final_chunk(512, 512)
            emit_final_chunk(1024, 128)

        # Pair iteration: mm1+exp slots of pair p interleaved with the mm2
        # passes + norm of pair p-1 (3 DR passes per slot, chunk-major).
        pairs = [(i, j) for j in range(3) for i in range(3)]
        prev = None
        for idx, (i, j) in enumerate(pairs):
            if idx == 0:
                emit_q(0)
                emit_k(0, 0)
                emit_k(0, 1)
            E = e_pool.tile([P, MT, NSLICE], fp8, tag="E")
            pa = None
            rt = None
            for ms in range(MT // 2):
                if ms % 2 == 0:
                    rt = runt_ps.tile([P, 512], f32, tag="runt")
                if prev is not None:
                    # mm2 of the previous pair first: ready PE work while the
                    # mm1 slot's PSUM WAR (exp three slots back) resolves.
                    pi, pj, pE = prev
                    c0, cw = CHUNKS[ms // 3]
                    for ks in range(3 * (ms % 3), 3 * (ms % 3) + 3):
                        if ks == 0:
                            pa = mm2_ps.tile([P, 512], f32, tag="mm2")
                        emit_mm2_pass(pa, pj, pE, ks, c0, cw)
                    emit_mm1_slot(E, i, j, ms, rt)
                    if ms % 3 == 2:
                        emit_norm(pa, pi, pj, c0, cw)
                    continue
                emit_mm1_slot(E, i, j, ms, rt)
                if idx == 0:
                    # prologue: spread remaining projections across slots
                    if ms == 0:
                        emit_q(1)
                    elif ms == 1:
                        emit_q(2)
                    elif 2 <= ms <= 7:
                        emit_vT(range(3 * (ms - 2), 3 * (ms - 2) + 3))
            if idx == 1:
                emit_k(1, 0)
            elif idx == 2:
                emit_k(1, 1)
            elif idx == 3:
                emit_k(2, 0)
            elif idx == 4:
                emit_k(2, 1)
            prev = (i, j, E)
        emit_mm2_norm_tail(prev[0], prev[1], prev[2])

    nc.compile()
    return nc


def _get_program():
    if "nc" not in _CACHE:
        _CACHE["nc"] = _build_program()
    return _CACHE["nc"]


def _host_prep(x, wq, wk, wv, wo):
    import ml_dtypes

    bf16 = ml_dtypes.bfloat16
    xf = np.ascontiguousarray(x.reshape(B, C, N), dtype=np.float32)
    # wq: [3, R, C] -> wqT: [C, 3, R] -> doubled on r -> [KT, P, 3, 2R]
    wqT = np.transpose(wq, (2, 0, 1))
    wqT2 = np.ascontiguousarray(
        np.concatenate([wqT, wqT], axis=-1).reshape(KT, P, 3, 2 * R)
    ).astype(bf16)
    wkT = np.transpose(wk, (2, 0, 1))
    wkT2 = np.ascontiguousarray(
        np.concatenate([wkT, wkT], axis=-1).reshape(KT, P, 3, 2 * R)
    ).astype(bf16)
    wvT = np.ascontiguousarray(np.transpose(wv, (2, 0, 1)).reshape(KT, P, 3, R)).astype(bf16)
    # wo: [C, 3R] -> woT[i] = wo[:, 64i:64(i+1)].T
    woT = np.ascontiguousarray(
        np.stack([wo[:, R * i : R * (i + 1)].T for i in range(3)])
    ).astype(bf16)
    in_maps = []
    for core in range(N_CORES):
        b, h = core // 2, core % 2
        xcore = xf[b].reshape(KT, P, N)
        xn32 = np.ascontiguousarray(xcore[:, :, h * NSLICE : (h + 1) * NSLICE])
        xn_half = 0.5 * xn32
        in_maps.append(
            {
                "xb": xcore.astype(bf16),
                "xnb": xn32.astype(bf16),
                "xn": xn_half,
                "wqT": wqT2,
                "wkT": wkT2,
                "wvT": wvT,
                "woT": woT,
            }
        )
    return in_maps


def kernel(x, wq, wk, wv, wo):
    global LAST_RESULTS
    from concourse.bass_utils import run_bass_kernel_spmd

    x = np.asarray(x)
    nc = _get_program()
    in_maps = _host_prep(
        x, np.asarray(wq), np.asarray(wk), np.asarray(wv), np.asarray(wo)
    )
    res = run_bass_kernel_spmd(nc, in_maps, core_ids=list(range(N_CORES)))
    LAST_RESULTS = res
    out = np.empty((B, C, N), np.float32)
    for core in range(N_CORES):
        b, h = core // 2, core % 2
        out[b][:, h * NSLICE : (h + 1) * NSLICE] = res.results[core]["y"].reshape(
            C, NSLICE
        )
    return out.reshape(B, C, H, W).astype(x.dtype, copy=False)


# revision 46
# speedup vs baseline: 1.0102x; 1.0102x over previous
"""NestedAttention Trainium2 kernel (v6).

Reference computation (per batch b):
  q_i = wq[i] @ x ; k_j = wk[j] @ x ; v_j = wv[j] @ x        (1x1 convs, r=64)
  for i: acc_i = sum_j softmax_m(q_i^T k_j / sqrt(r)) applied to v_j
  out = wo @ concat_i(acc_i) ; y = x * sigmoid(out)

Sharding: 8 cores = batch(4) x query-column-halves(2). Each core holds full
k/v (m = 2304 keys) and a 1152-wide slice of query columns n; no cross-core
communication is needed (softmax is over m, fully on-core).

Design (v6):
  * exp split between ScalarE (exact table exp) and VectorE (Schraudolph
    bit-trick: one tensor_scalar f32->int8 writes fp8e4m3 exp bits directly);
    softmax normalization cancels the sawtooth error (~5e-4 end-to-end).
  * mm2 (v @ E with a ones-block computing Z) in fp8 DoubleRow: E and vT are
    fp8e4m3, each pass contracts two m-tiles (9 passes), interleaved 3 per
    mm1 slot, chunk-major, one pair behind mm1.
  * logit PSUM: [128,1024] tiles x3 bufs (exactly 2 banks each -> 3-deep
    exp pipelining) + the 128-wide n-tail of 4 m-tiles batched into one
    shared bank, exp'd in one ScalarE op.
  * q/k stored duplicated across both partition halves (doubled projection
    weights, exp scale halved) - the K=128 matmul then needs no zero pad.
  * softmax denominator: partition-shifted Z copy (DVE or ScalarE), in-place
    reciprocal_approx_fast; the j>0 accumulate runs on GPSIMD.
"""

import math
import os
import numpy as np

B, C, H, W = 4, 256, 48, 48
N = H * W            # 2304 keys (m) per image
NSLICE = N // 2      # 1152 query columns (n) per core
R = 64               # reduced channels
P = 128
MT = N // P          # 18 m-tiles
KT = C // P          # 2 contraction tiles over channels
MAIN = 1024          # main logit block; 1152 = 1024 + 128 runt
CHUNKS = [(0, 512), (512, 512), (1024, 128)]  # mm2 n chunks, bank aligned
N_CORES = 8

# m-tiles whose main exp runs on the DVE (Schraudolph); rest on ScalarE.
# Split found by on-hardware search; balances ScalarE/VectorE occupancy.
DVE_EXP_MTS = frozenset(
    int(s)
    for s in os.environ.get("NESTED_DVE_MTS", "1,3,5,7,9,13,15").split(",")
    if s
)

NORM_MODE = os.environ.get("NESTED_NORM", "copy1")  # copy1 | acopy
ADD_ENGINE = os.environ.get("NESTED_ADD", "gpsimd")  # gpsimd | vector

# Schraudolph: fp8e4m3_bits = round(logit * scale * 8/ln2 + (7*8 - sigma)).
# q/k are duplicated across both partition halves so the K=128 dot product
# double-counts; the extra 0.5 compensates.
SCH_C1 = 0.5 * 0.125 * 8.0 / math.log(2.0)
SCH_C2 = 7.0 * 8.0 - 0.5
EXP_SCALE = 0.5 * 0.125

_CACHE = {}
LAST_RESULTS = None


def _build_program():
    from contextlib import ExitStack

    import concourse.bass as bass
    import concourse.tile as tile
    from concourse import bacc, mybir

    f32 = mybir.dt.float32
    bf16 = mybir.dt.bfloat16
    fp8 = mybir.dt.float8e4
    i8 = mybir.dt.int8
    Exp = mybir.ActivationFunctionType.Exp
    Tanh = mybir.ActivationFunctionType.Tanh
    mult = mybir.AluOpType.mult
    add = mybir.AluOpType.add
    DR = mybir.MatmulPerfMode.DoubleRow

    nc = bacc.Bacc("TRN2", target_bir_lowering=False, debug=False)
    xb_d = nc.declare_dram_parameter("xb", [KT, P, N], bf16, isOutput=False)
    xnb_d = nc.declare_dram_parameter("xnb", [KT, P, NSLICE], bf16, isOutput=False)
    xn_d = nc.declare_dram_parameter("xn", [KT, P, NSLICE], f32, isOutput=False)
    wqT_d = nc.declare_dram_parameter("wqT", [KT, P, 3, 2 * R], bf16, isOutput=False)
    wkT_d = nc.declare_dram_parameter("wkT", [KT, P, 3, 2 * R], bf16, isOutput=False)
    wvT_d = nc.declare_dram_parameter("wvT", [KT, P, 3, R], bf16, isOutput=False)
    woT_d = nc.declare_dram_parameter("woT", [3, R, C], bf16, isOutput=False)
    y_d = nc.declare_dram_parameter("y", [KT, P, NSLICE], f32, isOutput=True)

    with tile.TileContext(nc) as tc, ExitStack() as ctx:
        consts = ctx.enter_context(tc.tile_pool(name="consts", bufs=1))
        # PSUM: 3x2 banks (main logits) + 1 bank (runts) + 1 bank (mm2) = 8
        big_ps = ctx.enter_context(tc.tile_pool(name="big_ps", bufs=3, space="PSUM"))
        runt_ps = ctx.enter_context(tc.tile_pool(name="runt_ps", bufs=1, space="PSUM"))
        mm2_ps = ctx.enter_context(tc.tile_pool(name="mm2_ps", bufs=1, space="PSUM"))
        e_pool = ctx.enter_context(tc.tile_pool(name="e_pool", bufs=3))
        rb_pool = ctx.enter_context(tc.tile_pool(name="rb_pool", bufs=4))
        small = ctx.enter_context(tc.tile_pool(name="small", bufs=4))

        # ---- persistent SBUF state ----
        wqT_sb = consts.tile([P, KT, 3, 2 * R], bf16)
        nc.sync.dma_start(wqT_sb[:], wqT_d.rearrange("t p i r -> p t i r"))
        xnb_sb = consts.tile([P, KT, NSLICE], bf16)
        nc.sync.dma_start(xnb_sb[:], xnb_d.rearrange("t p m -> p t m"))
        wkT_sb = consts.tile([P, KT, 3, 2 * R], bf16)
        nc.sync.dma_start(wkT_sb[:], wkT_d.rearrange("t p i r -> p t i r"))
        x_sb = consts.tile([P, KT, N], bf16)
        nc.sync.dma_start(
            x_sb[:, :, 0:NSLICE], xb_d[:, :, 0:NSLICE].rearrange("t p m -> p t m")
        )
        nc.sync.dma_start(
            x_sb[:, :, NSLICE:N], xb_d[:, :, NSLICE:N].rearrange("t p m -> p t m")
        )
        wvT_sb = consts.tile([P, KT, 3, R], bf16)
        nc.sync.dma_start(wvT_sb[:], wvT_d.rearrange("t p i r -> p t i r"))
        xn_sb = consts.tile([P, KT, NSLICE], f32)
        nc.sync.dma_start(xn_sb[:], xn_d.rearrange("t p m -> p t m"))

        # Warm the ScalarE exp table set immediately so the ~2.7us table
        # load overlaps the input DMAs instead of stalling the first exp.
        warm = consts.tile([P, 520], bf16)
        nc.gpsimd.memset(warm[:], 0.0)
        nc.scalar.activation(
            warm[:, 0:8].bitcast(f32), warm[:, 8:16].bitcast(f32), Exp
        )

        woT_sb = []
        for i in range(3):
            w = consts.tile([P, C], bf16, tag=f"woT{i}")
            nc.gpsimd.memset(w[R:P, :], 0.0)
            nc.sync.dma_start(w[0:R, :], woT_d[i])
            woT_sb.append(w)

        # q/k live duplicated: partitions 0:64 and 64:128 hold the same values
        q_sb = consts.tile([P, 3, NSLICE], bf16)
        k_sb = consts.tile([P, 3, N], bf16)

        # vT buffer per m-tile (fp8): [vT_0 ones | vT_1 ones | vT_2 ones]
        vT_buf = consts.tile([P, MT, 384], fp8)
        for j in range(3):
            nc.gpsimd.memset(vT_buf[:, :, 128 * j + 64 : 128 * j + 128], 1.0)

        # acc_i accumulated in bf16; rows 64-127 zero (K=128 pad for final mm)
        acc = []
        for i in range(3):
            a = consts.tile([P, NSLICE], bf16, tag=f"acc{i}")
            nc.gpsimd.memset(a[R:P, :], 0.0)
            acc.append(a)

        # ---- projections (weights doubled -> output rows 0:64 == 64:128) ----
        def emit_proj(dst, w_sb, src_sb, idx3, src_off):
            ptm = big_ps.tile([P, MAIN], f32, tag="big")
            ptt = mm2_ps.tile([P, 512], f32, tag="mm2")
            for kt in range(KT):
                for c0, cw in ((0, 512), (512, 512)):
                    nc.tensor.matmul(
                        ptm[:, c0 : c0 + cw],
                        w_sb[:, kt, idx3, :],
                        src_sb[:, kt, src_off + c0 : src_off + c0 + cw],
                        start=(kt == 0),
                        stop=(kt == KT - 1),
                    )
                nc.tensor.matmul(
                    ptt[:, 0:128],
                    w_sb[:, kt, idx3, :],
                    src_sb[:, kt, src_off + 1024 : src_off + 1152],
                    start=(kt == 0),
                    stop=(kt == KT - 1),
                )
            nc.vector.tensor_copy(dst[:, 0:MAIN], ptm[:])
            nc.vector.tensor_copy(dst[:, MAIN:NSLICE], ptt[:, 0:128])

        def emit_q(i):
            emit_proj(q_sb[:, i, :], wqT_sb, xnb_sb, i, 0)

        def emit_k(j, half):
            emit_proj(
                k_sb[:, j, half * NSLICE : (half + 1) * NSLICE],
                wkT_sb,
                x_sb,
                j,
                half * NSLICE,
            )

        def emit_vT(mts):
            for mt in mts:
                pv = mm2_ps.tile([P, 512], f32, tag="mm2")
                for kt in range(KT):
                    nc.tensor.matmul(
                        pv[:, 0 : 3 * R],
                        x_sb[:, kt, mt * P : (mt + 1) * P],
                        wvT_sb[:, kt, :, :],
                        start=(kt == 0),
                        stop=(kt == KT - 1),
                    )
                base = vT_buf[:, mt, :]
                dst = bass.AP(
                    tensor=base.tensor,
                    offset=base.offset,
                    ap=[base.ap[0], [128, 3], [1, R]],
                )
                nc.vector.tensor_copy(
                    dst, pv[:, 0 : 3 * R].rearrange("p (j r) -> p j r", j=3)
                )

        # ---- attention pair pipeline ----
        def emit_exp_main(E, mt, pt):
            if mt in DVE_EXP_MTS:
                nc.vector.tensor_scalar(
                    E[:, mt, 0:MAIN].bitcast(i8), pt[:], SCH_C1, SCH_C2, mult, add
                )
            else:
                nc.scalar.activation(E[:, mt, 0:MAIN], pt[:], Exp, scale=EXP_SCALE)

        def emit_exp_runt(E, rt, mt0, nmt):
            nc.scalar.activation(
                E[:, mt0 : mt0 + nmt, MAIN:NSLICE],
                rt[:, 0 : nmt * 128].rearrange("p (g r) -> p g r", r=128),
                Exp,
                scale=EXP_SCALE,
            )

        def emit_mm1_slot(E, i, j, ms, rt):
            # Main matmuls + exps first; the runt matmuls go last so their
            # WAR on the single runt bank (previous group's ScalarE exp) has
            # a full slot of extra slack and never blocks the PE FIFO.
            for mt in (2 * ms, 2 * ms + 1):
                pt = big_ps.tile([P, MAIN], f32, tag="big")
                for c0, cw in ((0, 512), (512, 512)):
                    nc.tensor.matmul(
                        pt[:, c0 : c0 + cw],
                        k_sb[:, j, mt * P : (mt + 1) * P],
                        q_sb[:, i, c0 : c0 + cw],
                        start=True,
                        stop=True,
                    )
                emit_exp_main(E, mt, pt)
            for mt in (2 * ms, 2 * ms + 1):
                r0 = 128 * (mt % 4)
                nc.tensor.matmul(
                    rt[:, r0 : r0 + 128],
                    k_sb[:, j, mt * P : (mt + 1) * P],
                    q_sb[:, i, MAIN:NSLICE],
                    start=True,
                    stop=True,
                )
                if mt % 4 == 3:
                    emit_exp_runt(E, rt, mt - 3, 4)
                elif mt == MT - 1:
                    emit_exp_runt(E, rt, mt - 1, 2)

        po_m = [None, None]
        po_t = [None]

        def emit_final_chunk(c0, cw):
            for mtile in range(KT):
                dst = (
                    po_m[mtile][:, c0 : c0 + cw]
                    if c0 < MAIN
                    else po_t[0][:, 128 * mtile : 128 * mtile + cw]
                )
                for i in range(3):
                    nc.tensor.matmul(
                        dst,
                        woT_sb[i][:, mtile * P : (mtile + 1) * P],
                        acc[i][:, c0 : c0 + cw],
                        start=(i == 0),
                        stop=(i == 2),
                    )
            for mtile in range(KT):
                src = (
                    po_m[mtile][:, c0 : c0 + cw]
                    if c0 < MAIN
                    else po_t[0][:, 128 * mtile : 128 * mtile + cw]
                )
                # sigmoid(x) = 0.5*(1 + tanh(x/2)); tanh shares the ACT table
                # set with exp (no mid-kernel table switch). xn_sb holds x/2.
                sig = small.tile([P, 512], f32, tag="sig")
                nc.scalar.activation(sig[:, 0:cw], src, Tanh, scale=0.5)
                y_sb = small.tile([P, 512], f32, tag="ysb")
                nc.vector.scalar_tensor_tensor(
                    y_sb[:, 0:cw],
                    sig[:, 0:cw],
                    1.0,
                    xn_sb[:, mtile, c0 : c0 + cw],
                    add,
                    mult,
                )
                nc.sync.dma_start(y_d[mtile][:, c0 : c0 + cw], y_sb[:, 0:cw])

        def emit_mm2_pass(pa, j, E, ks, c0, cw):
            nc.tensor.matmul(
                pa[:, 0:cw],
                vT_buf[:, 2 * ks : 2 * ks + 2, 128 * j : 128 * (j + 1)],
                E[:, 2 * ks : 2 * ks + 2, c0 : c0 + cw],
                start=(ks == 0),
                stop=(ks == MT // 2 - 1),
                perf_mode=DR,
            )

        def emit_norm(pa, i, j, c0, cw, last=False):
            rb = rb_pool.tile([R, 512], f32, tag="rb")
            # j==2 norms run while ScalarE has slack (the tail region):
            # route their Z copy through ScalarE to shorten the DVE chain.
            if NORM_MODE == "acopy" or j == 2:
                nc.scalar.copy(rb[:, 0:cw], pa[R:P, 0:cw])
            else:
                nc.vector.tensor_copy(rb[:, 0:cw], pa[R:P, 0:cw])
            nc.vector.reciprocal_approx_fast(rb[:, 0:cw], rb[:, 0:cw])
            if j == 0:
                nc.vector.tensor_tensor(
                    acc[i][0:R, c0 : c0 + cw], pa[0:R, 0:cw], rb[:, 0:cw], mult
                )
            else:
                tmp = small.tile([R, 512], bf16, tag="tmp")
                nc.vector.tensor_tensor(
                    tmp[:, 0:cw], pa[0:R, 0:cw], rb[:, 0:cw], mult
                )
                eng = nc.gpsimd if ADD_ENGINE == "gpsimd" else nc.vector
                eng.tensor_tensor(
                    acc[i][0:R, c0 : c0 + cw],
                    acc[i][0:R, c0 : c0 + cw],
                    tmp[:, 0:cw],
                    add,
                )
            if last:
                emit_final_chunk(c0, cw)

        def emit_mm2_norm_tail(i, j, E):
            # Epilogue for the last pair: keep the PE dense by giving each
            # chunk its own PSUM bank (big pool + mm2 bank are free now) so
            # all 27 DR passes run back-to-back while norms/finals overlap.
            pa0 = big_ps.tile([P, MAIN], f32, tag="big", name="pa_ep0")
            for ks in range(MT // 2):
                emit_mm2_pass(pa0, j, E, ks, 0, 512)
            emit_norm(pa0, i, j, 0, 512)
            pa1 = big_ps.tile([P, MAIN], f32, tag="big", name="pa_ep1")
            for ks in range(MT // 2):
                emit_mm2_pass(pa1, j, E, ks, 512, 512)
            emit_norm(pa1, i, j, 512, 512)
            # chunk-2 accumulator and the final's n-tail share the runt bank,
            # which frees early (no dependency on the previous pair's norms).
            rt_ep = runt_ps.tile([P, 512], f32, tag="runt", name="rt_ep")
            pa2 = rt_ep
            for ks in range(MT // 2):
                emit_mm2_pass(pa2, j, E, ks, 1024, 128)
            emit_norm(pa2, i, j, 1024, 128)
            po_m[0] = big_ps.tile([P, MAIN], f32, tag="big", name="po0")
            po_m[1] = big_ps.tile([P, MAIN], f32, tag="big", name="po1")
            po_t[0] = rt_ep[:, 128:512]
            emit_final_chunk(0, 512)
            emit_final_chunk(512, 512)
            emit_final_chunk(1024, 128)

        # Pair iteration: mm1+exp slots of pair p interleaved with the mm2
        # passes + norm of pair p-1 (3 DR passes per slot, chunk-major).
        pairs = [(i, j) for j in range(3) for i in range(3)]
        prev = None
        for idx, (i, j) in enumerate(pairs):
            if idx == 0:
                emit_q(0)
                emit_k(0, 0)
                emit_k(0, 1)
            E = e_pool.tile([P, MT, NSLICE], fp8, tag="E")
            pa = None
            rt = None
            for ms in range(MT // 2):
                if ms % 2 == 0:
                    rt = runt_ps.tile([P, 512], f32, tag="runt")
                if prev is not None:
                    # mm2 of the previous pair first: ready PE work while the
                    # mm1 slot's PSUM WAR (exp three slots back) resolves.
                    pi, pj, pE = prev
                    c0, cw = CHUNKS[ms // 3]
                    for ks in range(3 * (ms % 3), 3 * (ms % 3) + 3):
                        if ks == 0:
                            pa = mm2_ps.tile([P, 512], f32, tag="mm2")
                        emit_mm2_pass(pa, pj, pE, ks, c0, cw)
                    emit_mm1_slot(E, i, j, ms, rt)
                    if ms % 3 == 2:
                        emit_norm(pa, pi, pj, c0, cw)
                    continue
                emit_mm1_slot(E, i, j, ms, rt)
                if idx == 0:
                    # prologue: spread remaining projections across slots
                    if ms == 0:
                        emit_q(1)
                    elif ms == 1:
                        emit_q(2)
                    elif 2 <= ms <= 7:
                        emit_vT(range(3 * (ms - 2), 3 * (ms - 2) + 3))
            if idx == 1:
                emit_k(1, 0)
                emit_k(1, 1)
            elif idx == 3:
                emit_k(2, 0)
                emit_k(2, 1)
            prev = (i, j, E)
        emit_mm2_norm_tail(prev[0], prev[1], prev[2])

    nc.compile()
    return nc


def _get_program():
    if "nc" not in _CACHE:
        _CACHE["nc"] = _build_program()
    return _CACHE["nc"]


def _host_prep(x, wq, wk, wv, wo):
    import ml_dtypes

    bf16 = ml_dtypes.bfloat16
    xf = np.ascontiguousarray(x.reshape(B, C, N), dtype=np.float32)
    # wq: [3, R, C] -> wqT: [C, 3, R] -> doubled on r -> [KT, P, 3, 2R]
    wqT = np.transpose(wq, (2, 0, 1))
    wqT2 = np.ascontiguousarray(
        np.concatenate([wqT, wqT], axis=-1).reshape(KT, P, 3, 2 * R)
    ).astype(bf16)
    wkT = np.transpose(wk, (2, 0, 1))
    wkT2 = np.ascontiguousarray(
        np.concatenate([wkT, wkT], axis=-1).reshape(KT, P, 3, 2 * R)
    ).astype(bf16)
    wvT = np.ascontiguousarray(np.transpose(wv, (2, 0, 1)).reshape(KT, P, 3, R)).astype(bf16)
    # wo: [C, 3R] -> woT[i] = wo[:, 64i:64(i+1)].T
    woT = np.ascontiguousarray(
        np.stack([wo[:, R * i : R * (i + 1)].T for i in range(3)])
    ).astype(bf16)
    in_maps = []
    for core in range(N_CORES):
        b, h = core // 2, core % 2
        xcore = xf[b].reshape(KT, P, N)
        xn32 = np.ascontiguousarray(xcore[:, :, h * NSLICE : (h + 1) * NSLICE])
        xn_half = 0.5 * xn32
        in_maps.append(
            {
                "xb": xcore.astype(bf16),
                "xnb": xn32.astype(bf16),
                "xn": xn_half,
                "wqT": wqT2,
                "wkT": wkT2,
                "wvT": wvT,
                "woT": woT,
            }
        )
    return in_maps


def kernel(x, wq, wk, wv, wo):
    global LAST_RESULTS
    from concourse.bass_utils import run_bass_kernel_spmd

    x = np.asarray(x)
    nc = _get_program()
    in_maps = _host_prep(
        x, np.asarray(wq), np.asarray(wk), np.asarray(wv), np.asarray(wo)
    )
    res = run_bass_kernel_spmd(nc, in_maps, core_ids=list(range(N_CORES)))
    LAST_RESULTS = res
    out = np.empty((B, C, N), np.float32)
    for core in range(N_CORES):
        b, h = core // 2, core % 2
        out[b][:, h * NSLICE : (h + 1) * NSLICE] = res.results[core]["y"].reshape(
            C, NSLICE
        )
    return out.reshape(B, C, H, W).astype(x.dtype, copy=False)
